# Initial kernel scaffold
#
"""AGCRN cell on 8 TRN2 NeuronCores — node-sharded SPMD Bass kernel.

N=2048 nodes sharded 256/core. Two SPMD launches:
  phase 1 (gate, Co=128):  z_r = sigmoid(avwgcn(concat(X,state)));  out zs=z*state, r
  phase 2 (update, Co=64): hc = tanh(avwgcn(concat(X,zs_full)));    out h = r*state+(1-r)*hc
Host only reshapes/shards between phases.

Layout tricks (no PE transposes anywhere):
  - adjacency numerator computed transposed T[m,n] = max(exp(Ef@Eo.T),1)
    (exp(relu(x)) == max(exp(x),1)); row-softmax denominator den[n] via
    ones-matmul; 1/den folded back into T columns via an outer-product
    broadcast, so graph-conv output needs no further normalization.
  - graph conv emitted directly in apply-ready layout: SXT[c, n, b]
    (lhsT = XS m-chunk column-slice per b, rhs = T m-chunk).
  - per-node weights W[n] = sum_d Eo[n,d]*Wp[d] generated on PE in
    [n, (k,i,o)] layout, bounced through DRAM, DMA'd back per node as
    [i, co] stationary tiles for the per-node apply matmuls.
"""

import numpy as np
import ml_dtypes

from concourse import bacc, mybir
from concourse import tile
from concourse.bass_utils import run_bass_kernel_spmd

NCORES = 8
N, B, CIN, HID, D = 2048, 16, 2, 64, 16
CI = CIN + HID          # 66
NO = N // NCORES        # 256 own nodes per core
NT = NO // 128          # 2 node-tiles
MC = N // 128           # 16 m-chunks
BC = B * CI             # 1056

F32 = mybir.dt.float32
F32R = mybir.dt.float32r
BF16 = mybir.dt.bfloat16
AF = mybir.ActivationFunctionType
ALU = mybir.AluOpType


def build_phase(co, act_func, second):
    nc = bacc.Bacc("TRN2", target_bir_lowering=False, debug=False,
                   num_devices=NCORES)
    eoT = nc.dram_tensor("EoT", [D, NO], F32R, kind="ExternalInput")
    efT = nc.dram_tensor("EfT", [D, N], F32R, kind="ExternalInput")
    wf = nc.dram_tensor("Wf", [D, 2 * CI * co], F32R, kind="ExternalInput")
    bp = nc.dram_tensor("bp", [D, co], F32R, kind="ExternalInput")
    xs = nc.dram_tensor("XS", [N, BC], BF16, kind="ExternalInput")
    xt = nc.dram_tensor("XT", [CI, NO, B], BF16, kind="ExternalInput")
    st = nc.dram_tensor("ST", [HID, NO * B], F32, kind="ExternalInput")
    if second:
        rin = nc.dram_tensor("R", [HID, NO * B], F32, kind="ExternalInput")
        hout = nc.dram_tensor("h", [HID, NO * B], F32, kind="ExternalOutput")
    else:
        zsout = nc.dram_tensor("zs", [HID, NO * B], F32, kind="ExternalOutput")
        rout = nc.dram_tensor("r", [HID, NO * B], F32, kind="ExternalOutput")

    nwf = 2 * CI * co                    # flattened (k,i,o) length
    rows_per = 4 if co == 128 else 6     # W rows per gen chunk; 132 % rows == 0
    chunk = rows_per * co                # 512 (gate) / 384 (upd), both >= 256
    ngc = (2 * CI) // rows_per           # 33 / 22 chunks

    with tile.TileContext(nc) as tc:
        with (
            tc.tile_pool(name="psG", bufs=2, space="PSUM") as psG,
            tc.tile_pool(name="psT", bufs=2, space="PSUM") as psT,
            tc.tile_pool(name="psC", bufs=1, space="PSUM") as psC,
            tc.tile_pool(name="psD", bufs=2, space="PSUM") as psD,
            tc.tile_pool(name="sb", bufs=1) as sb,
            tc.tile_pool(name="sbs", bufs=3) as sbs,
            tc.tile_pool(name="wt", bufs=8) as wtp,
            tc.tile_pool(name="dram", bufs=1, space="DRAM") as dram,
        ):
            # ---- small SBUF-resident inputs
            eoT_s = sb.tile([D, NO], F32R, tag="eoT")
            nc.sync.dma_start(out=eoT_s[:], in_=eoT[:])
            efT_s = sb.tile([D, N], F32R, tag="efT")
            nc.sync.dma_start(out=efT_s[:], in_=efT[:])
            bp_s = sb.tile([D, co], F32R, tag="bp")
            nc.sync.dma_start(out=bp_s[:], in_=bp[:])
            xt_s = sb.tile([CI, NO, B], BF16, tag="xt")
            nc.sync.dma_start(out=xt_s[:], in_=xt[:])
            st_s = sb.tile([HID, NO * B], F32, tag="st")
            nc.sync.dma_start(out=st_s[:], in_=st[:])
            if second:
                r_s = sb.tile([HID, NO * B], F32, tag="r")
                nc.sync.dma_start(out=r_s[:], in_=rin[:])
            ones_c = sb.tile([128, 1], BF16, tag="ones_c")
            nc.vector.memset(ones_c[:], 1.0)
            ones_r = sb.tile([1, 128], F32, tag="ones_r")
            nc.vector.memset(ones_r[:], 1.0)

            # ---- biasT [co, NO] = bp.T @ Eo.T
            bps = psG.tile([128, 512], F32, tag="gen")
            nc.tensor.matmul(bps[:co, :NO], bp_s[:], eoT_s[:],
                             start=True, stop=True)
            biasT = sb.tile([co, NO], F32, tag="biasT")
            nc.vector.tensor_copy(biasT[:], bps[:co, :NO])

            # ---- per-node weight gen:  W[n,(k,i,o)] = sum_d Eo[n,d] Wf[d,:]
            wdram = dram.tile([NO, 2 * CI, co], BF16, tag="wdram")
            for t in range(NT):
                for c in range(ngc):
                    lo = c * chunk
                    wfc = sbs.tile([D, chunk], F32R, tag="wfc")
                    nc.sync.dma_start(out=wfc[:], in_=wf[:, lo:lo + chunk])
                    gp = psG.tile([128, 512], F32, tag="gen")
                    nc.tensor.matmul(gp[:, :chunk],
                                     eoT_s[:, t * 128:(t + 1) * 128],
                                     wfc[:], start=True, stop=True)
                    wc = sbs.tile([128, 512], BF16, tag="wc")
                    nc.vector.tensor_copy(wc[:, :chunk], gp[:, :chunk])
                    nc.sync.dma_start(
                        out=wdram[t * 128:(t + 1) * 128,
                                  c * rows_per:(c + 1) * rows_per, :],
                        in_=wc[:, :chunk].rearrange(
                            "p (r o) -> p r o", r=rows_per))

            # ---- T[m,n] = max(exp(Ef@Eo.T), 1)  (transposed layout, bf16)
            tch = []
            for m in range(MC):
                tp = psT.tile([128, NO], F32, tag="tgen")
                nc.tensor.matmul(tp[:], efT_s[:, m * 128:(m + 1) * 128],
                                 eoT_s[:], start=True, stop=True)
                ts = sb.tile([128, NO], BF16, tag=f"T{m}")
                nc.scalar.activation(ts[:], tp[:], AF.Exp)
                nc.vector.tensor_scalar_max(ts[:], ts[:], 1.0)
                tch.append(ts)

            # ---- den[n] = sum_m T[m,n]; fold 1/den into T
            dp = psC.tile([1, NO], F32, tag="den")
            for m in range(MC):
                nc.tensor.matmul(dp[:], ones_c[:], tch[m][:],
                                 start=(m == 0), stop=(m == MC - 1))
            rrow = sb.tile([1, NO], F32, tag="rrow")
            nc.vector.reciprocal(rrow[:], dp[:])
            rbp = psC.tile([128, NO], F32, tag="rb")
            nc.tensor.matmul(rbp[:], ones_r[:], rrow[:], start=True, stop=True)
            for m in range(MC):
                nc.vector.tensor_tensor(tch[m][:], tch[m][:], rbp[:],
                                        op=ALU.mult)

            # ---- resident XS chunks (graph-conv moving operand source)
            xsc = []
            for m in range(MC):
                xc_ = sb.tile([128, BC], BF16, tag=f"XS{m}")
                nc.sync.dma_start(out=xc_[:], in_=xs[m * 128:(m + 1) * 128, :])
                xsc.append(xc_)

            # ---- graph conv, transposed: SXT[c, n, b], accumulate over m
            sxt = sb.tile([CI, NO, B], BF16, tag="sxt")
            for b in range(B):
                cp = psD.tile([CI, NO], F32, tag="conv")
                for m in range(MC):
                    nc.tensor.matmul(cp[:], xsc[m][:, b * CI:(b + 1) * CI],
                                     tch[m][:], start=(m == 0),
                                     stop=(m == MC - 1))
                nc.vector.tensor_copy(sxt[:, :, b], cp[:])

            # ---- per-node apply + fused bias+activation
            zr = sb.tile([co, NO * B], F32, tag="zr")
            for n in range(NO):
                ap = psG.tile([128, 512], F32, tag="gen")
                w0 = wtp.tile([CI, co], BF16, tag="w0")
                nc.sync.dma_start(out=w0[:], in_=wdram[n, 0:CI, :])
                w1 = wtp.tile([CI, co], BF16, tag="w1")
                nc.sync.dma_start(out=w1[:], in_=wdram[n, CI:2 * CI, :])
                nc.tensor.matmul(ap[:co, :B], w0[:], xt_s[:, n, :],
                                 start=True, stop=False)
                nc.tensor.matmul(ap[:co, :B], w1[:], sxt[:, n, :],
                                 start=False, stop=True)
                nc.scalar.activation(zr[:, n * B:(n + 1) * B], ap[:co, :B],
                                     act_func, bias=biasT[:, n:n + 1])

            # ---- postlude + outputs
            if second:
                tmp = sb.tile([HID, NO * B], F32, tag="tmp")
                nc.vector.tensor_sub(tmp[:], st_s[:], zr[:])
                nc.vector.tensor_mul(tmp[:], tmp[:], r_s[:])
                nc.vector.tensor_add(tmp[:], tmp[:], zr[:])
                nc.sync.dma_start(out=hout[:], in_=tmp[:])
            else:
                zs_s = sb.tile([HID, NO * B], F32, tag="zs")
                nc.vector.tensor_mul(zs_s[:], zr[:HID, :], st_s[:])
                nc.sync.dma_start(out=zsout[:], in_=zs_s[:])
                nc.sync.dma_start(out=rout[:], in_=zr[HID:, :])
    return nc


_CACHE = {}


def _phases():
    if "p" not in _CACHE:
        nc1 = build_phase(2 * HID, AF.Sigmoid, False)
        nc1.finalize()
        nc2 = build_phase(HID, AF.Tanh, True)
        nc2.finalize()
        _CACHE["p"] = (nc1, nc2)
    return _CACHE["p"]


def kernel(X, state, E, gate_W, gate_b, upd_W, upd_b):
    X = np.asarray(X, np.float32)
    state = np.asarray(state, np.float32)
    E = np.asarray(E, np.float32)
    bf = ml_dtypes.bfloat16
    nc1, nc2 = _phases()
    cores = list(range(NCORES))

    efT = np.ascontiguousarray(E.T)                       # [16, 2048]
    xin = np.concatenate([X, state], -1)                  # [B, N, 66]
    xs1 = np.ascontiguousarray(
        xin.transpose(1, 0, 2).reshape(N, BC)).astype(bf)
    wf1 = np.ascontiguousarray(np.asarray(gate_W, np.float32).reshape(D, -1))
    wf2 = np.ascontiguousarray(np.asarray(upd_W, np.float32).reshape(D, -1))
    bp1 = np.asarray(gate_b, np.float32)
    bp2 = np.asarray(upd_b, np.float32)

    in1, stl = [], []
    for c in cores:
        s = slice(c * NO, (c + 1) * NO)
        eoT = np.ascontiguousarray(E[s].T)                # [16, 256]
        xtc = np.ascontiguousarray(
            xin[:, s].transpose(2, 1, 0)).astype(bf)      # [66, 256, 16]
        stc = np.ascontiguousarray(
            state[:, s].transpose(2, 1, 0).reshape(HID, NO * B))
        stl.append(stc)
        in1.append(dict(EoT=eoT, EfT=efT, Wf=wf1, bp=bp1, XS=xs1,
                        XT=xtc, ST=stc))
    res1 = run_bass_kernel_spmd(nc1, in1, cores).results

    # zs_full [m, b, c] from per-core zs [HID, NO*B] == (c, n, b)
    zs_all = np.concatenate(
        [r["zs"].reshape(HID, NO, B).transpose(1, 2, 0) for r in res1], 0)
    xc = np.concatenate([X.transpose(1, 0, 2), zs_all], 2)  # [N, B, 66]
    xs2 = np.ascontiguousarray(xc.reshape(N, BC)).astype(bf)
    in2 = []
    for c in cores:
        s = slice(c * NO, (c + 1) * NO)
        xtc = np.ascontiguousarray(xc[s].transpose(2, 0, 1)).astype(bf)
        in2.append(dict(EoT=in1[c]["EoT"], EfT=efT, Wf=wf2, bp=bp2,
                        XS=xs2, XT=xtc, ST=stl[c], R=res1[c]["r"]))
    res2 = run_bass_kernel_spmd(nc2, in2, cores).results

    h = np.concatenate(
        [r["h"].reshape(HID, NO, B).transpose(2, 1, 0) for r in res2], 1)
    return np.ascontiguousarray(h, np.float32)



# revision 10
# speedup vs baseline: 1.0227x; 1.0227x over previous
"""AGCRN cell on 8 TRN2 NeuronCores — node-sharded SPMD Bass kernel.

N=2048 nodes sharded 256/core. Two SPMD launches:
  phase 1 (gate, Co=128):  z_r = sigmoid(avwgcn(concat(X,state)));  out zs=z*state, r
  phase 2 (update, Co=64): hc = tanh(avwgcn(concat(X,zs_full)));    out h = r*state+(1-r)*hc
Host only reshapes/shards between phases.

Layout tricks (no PE transposes anywhere):
  - adjacency numerator computed transposed T[m,n] = max(exp(Ef@Eo.T),1);
    row-softmax denominator via ones-matmul; 1/den folded back into T columns.
  - graph conv emitted directly in apply-ready layout: SXT[c, n, b].
  - per-node weights W[n] = sum_d Eo[n,d]*Wp[d] generated on PE in
    [n, (k,i,o)] layout with a BIAS ROW folded in (input carries a ones
    row), bounced through DRAM, then DMA'd back in a few large strided
    transfers as [ki, n, co] stationary tiles (not 512 per-node DMAs).
  - activations batched 32 nodes per call instead of per-node.
  - DMA issue spread across sync/scalar/vector/gpsimd queues.
"""

import numpy as np
import ml_dtypes

from concourse import bacc, mybir
from concourse import tile
from concourse.bass_utils import run_bass_kernel_spmd

NCORES = 8
N, B, CIN, HID, D = 2048, 16, 2, 64, 16
CI = CIN + HID          # 66
CIB = CI + 1            # 67 = 66 channels + ones/bias row
ROWS = 2 * CI + 1       # 133 weight rows per node: k0(66)+bias(1)+k1(66)
NO = N // NCORES        # 256 own nodes per core
MC = N // 128           # 16 m-chunks
BC = B * CI             # 1056
NSUB = 64               # nodes per W-readback subtile
NBLK = 32               # nodes per batched activation

F32 = mybir.dt.float32
F32R = mybir.dt.float32r
BF16 = mybir.dt.bfloat16
AF = mybir.ActivationFunctionType
ALU = mybir.AluOpType


def build_phase(co, act_func, second):
    nc = bacc.Bacc("TRN2", target_bir_lowering=False, debug=False,
                   num_devices=NCORES)
    eoT = nc.dram_tensor("EoT", [D, NO], F32R, kind="ExternalInput")
    efT = nc.dram_tensor("EfT", [D, N], F32R, kind="ExternalInput")
    wf = nc.dram_tensor("Wf", [D, ROWS * co], F32R, kind="ExternalInput")
    xs = nc.dram_tensor("XS", [N, BC], BF16, kind="ExternalInput")
    xt = nc.dram_tensor("XT", [CIB, NO, B], BF16, kind="ExternalInput")
    st = nc.dram_tensor("ST", [HID, NO * B], F32, kind="ExternalInput")
    if second:
        rin = nc.dram_tensor("R", [HID, NO * B], F32, kind="ExternalInput")
        hout = nc.dram_tensor("h", [HID, NO * B], F32, kind="ExternalOutput")
    else:
        zsout = nc.dram_tensor("zs", [HID, NO * B], F32, kind="ExternalOutput")
        rout = nc.dram_tensor("r", [HID, NO * B], F32, kind="ExternalOutput")

    # ragged gen chunks over the 133 weight rows, <=512 free per matmul
    rows_per = 512 // co
    gchunks = []
    r0 = 0
    while r0 < ROWS:
        nr = min(rows_per, ROWS - r0)
        gchunks.append((r0, nr))
        r0 += nr

    with tile.TileContext(nc) as tc:
        with (
            tc.tile_pool(name="psG", bufs=2, space="PSUM") as psG,
            tc.tile_pool(name="psT", bufs=2, space="PSUM") as psT,
            tc.tile_pool(name="psC", bufs=1, space="PSUM") as psC,
            tc.tile_pool(name="psD", bufs=2, space="PSUM") as psD,
            tc.tile_pool(name="sb", bufs=1) as sb,
            tc.tile_pool(name="sbs", bufs=3) as sbs,
            tc.tile_pool(name="wt", bufs=2) as wtp,
            tc.tile_pool(name="dram", bufs=1, space="DRAM") as dram,
        ):
            qs = [nc.sync, nc.scalar, nc.gpsimd]

            # ---- small SBUF-resident inputs (spread queues)
            eoT_s = sb.tile([D, NO], F32R, tag="eoT")
            nc.sync.dma_start(out=eoT_s[:], in_=eoT[:])
            efT_s = sb.tile([D, N], F32R, tag="efT")
            nc.sync.dma_start(out=efT_s[:], in_=efT[:])
            xt_s = sb.tile([CIB, NO, B], BF16, tag="xt")
            nc.scalar.dma_start(out=xt_s[:], in_=xt[:])
            st_s = sb.tile([HID, NO * B], F32, tag="st")
            nc.gpsimd.dma_start(out=st_s[:], in_=st[:])
            if second:
                r_s = sb.tile([HID, NO * B], F32, tag="r")
                nc.gpsimd.dma_start(out=r_s[:], in_=rin[:])
            ones_c = sb.tile([128, 1], BF16, tag="ones_c")
            nc.vector.memset(ones_c[:], 1.0)
            ones_r = sb.tile([1, 128], F32, tag="ones_r")
            nc.vector.memset(ones_r[:], 1.0)

            # ---- per-node weight gen:  W[n,(r)] = sum_d Eo[n,d] Wf[d,:]
            wdram = dram.tile([NO, ROWS, co], BF16, tag="wdram")
            qi = 0
            for t in range(2):
                for (r0, nr) in gchunks:
                    chunk = nr * co
                    wfc = sbs.tile([D, 512], F32R, tag="wfc")
                    qs[qi % 3].dma_start(
                        out=wfc[:, :chunk], in_=wf[:, r0 * co:r0 * co + chunk])
                    gp = psG.tile([128, 512], F32, tag="gen")
                    nc.tensor.matmul(gp[:, :chunk],
                                     eoT_s[:, t * 128:(t + 1) * 128],
                                     wfc[:, :chunk], start=True, stop=True)
                    wc = sbs.tile([128, 512], BF16, tag="wc")
                    if qi % 2 == 0:
                        nc.vector.tensor_copy(wc[:, :chunk], gp[:, :chunk])
                    else:
                        nc.scalar.activation(wc[:, :chunk], gp[:, :chunk],
                                             AF.Copy)
                    qs[(qi + 1) % 3].dma_start(
                        out=wdram[t * 128:(t + 1) * 128, r0:r0 + nr, :],
                        in_=wc[:, :chunk].rearrange(
                            "p (r o) -> p r o", r=nr))
                    qi += 1

            # ---- T[m,n] = max(exp(Ef@Eo.T), 1)  (transposed layout, bf16)
            tch = []
            for m in range(MC):
                tp = psT.tile([128, NO], F32, tag="tgen")
                nc.tensor.matmul(tp[:], efT_s[:, m * 128:(m + 1) * 128],
                                 eoT_s[:], start=True, stop=True)
                ts = sb.tile([128, NO], BF16, tag=f"T{m}")
                nc.scalar.activation(ts[:], tp[:], AF.Exp)
                nc.vector.tensor_scalar_max(ts[:], ts[:], 1.0)
                tch.append(ts)

            # ---- den[n] = sum_m T[m,n]; fold 1/den into T
            dp = psC.tile([1, NO], F32, tag="den")
            for m in range(MC):
                nc.tensor.matmul(dp[:], ones_c[:], tch[m][:],
                                 start=(m == 0), stop=(m == MC - 1))
            rrow = sb.tile([1, NO], F32, tag="rrow")
            nc.vector.reciprocal(rrow[:], dp[:])
            rbp = psC.tile([128, NO], F32, tag="rb")
            nc.tensor.matmul(rbp[:], ones_r[:], rrow[:], start=True, stop=True)
            for m in range(MC):
                nc.vector.tensor_tensor(tch[m][:], tch[m][:], rbp[:],
                                        op=ALU.mult)

            # ---- resident XS chunks (graph-conv moving operand source)
            xsc = []
            for m in range(MC):
                xc_ = sb.tile([128, BC], BF16, tag=f"XS{m}")
                qs[m % 3].dma_start(out=xc_[:], in_=xs[m * 128:(m + 1) * 128, :])
                xsc.append(xc_)

            # ---- graph conv, transposed: SXT[c, n, b], accumulate over m
            sxt = sb.tile([CI, NO, B], BF16, tag="sxt")
            for b in range(B):
                cp = psD.tile([CI, NO], F32, tag="conv")
                for m in range(MC):
                    nc.tensor.matmul(cp[:], xsc[m][:, b * CI:(b + 1) * CI],
                                     tch[m][:], start=(m == 0),
                                     stop=(m == MC - 1))
                if b % 2 == 0:
                    nc.vector.tensor_copy(sxt[:, :, b], cp[:])
                else:
                    nc.scalar.activation(sxt[:, :, b], cp[:], AF.Copy)

            # ---- W readback: a few large strided DMAs (not per-node)
            # wdram [NO, ROWS, co] -> per subtile s: W0 [67, NSUB, co],
            # W1 [66, NSUB, co]
            zr = sb.tile([co, NO * B], F32, tag="zr")
            for s in range(NO // NSUB):
                lo = s * NSUB
                w0 = wtp.tile([CIB, NSUB, co], BF16, tag="w0")
                nc.sync.dma_start(
                    out=w0[:],
                    in_=wdram[lo:lo + NSUB, 0:CIB, :].rearrange(
                        "n r o -> r n o"))
                w1 = wtp.tile([CI, NSUB, co], BF16, tag="w1")
                nc.scalar.dma_start(
                    out=w1[:],
                    in_=wdram[lo:lo + NSUB, CIB:ROWS, :].rearrange(
                        "n r o -> r n o"))
                # ---- per-node apply, batched activation per 32 nodes
                for blk in range(NSUB // NBLK):
                    ap = psG.tile([128, 512], F32, tag="gen")
                    for j in range(NBLK):
                        nn = blk * NBLK + j
                        n = lo + nn
                        nc.tensor.matmul(ap[:co, j * B:(j + 1) * B],
                                         w0[:, nn, :], xt_s[:, n, :],
                                         start=True, stop=False)
                        nc.tensor.matmul(ap[:co, j * B:(j + 1) * B],
                                         w1[:, nn, :], sxt[:, n, :],
                                         start=False, stop=True)
                    nc.scalar.activation(
                        zr[:, (lo + blk * NBLK) * B:
                           (lo + blk * NBLK + NBLK) * B],
                        ap[:co, :], act_func)

            # ---- postlude + outputs
            if second:
                tmp = sb.tile([HID, NO * B], F32, tag="tmp")
                nc.vector.tensor_sub(tmp[:], st_s[:], zr[:])
                nc.vector.tensor_mul(tmp[:], tmp[:], r_s[:])
                nc.vector.tensor_add(tmp[:], tmp[:], zr[:])
                nc.sync.dma_start(out=hout[:], in_=tmp[:])
            else:
                zs_s = sb.tile([HID, NO * B], F32, tag="zs")
                nc.vector.tensor_mul(zs_s[:], zr[:HID, :], st_s[:])
                nc.sync.dma_start(out=zsout[:], in_=zs_s[:])
                nc.scalar.dma_start(out=rout[:], in_=zr[HID:, :])
    return nc


_CACHE = {}
TRACE = False
LAST_EXEC_NS = None
LAST_PHASE_NS = []
LAST_TRACE = []


def _phases():
    if "p" not in _CACHE:
        nc1 = build_phase(2 * HID, AF.Sigmoid, False)
        nc1.finalize()
        nc2 = build_phase(HID, AF.Tanh, True)
        nc2.finalize()
        _CACHE["p"] = (nc1, nc2)
    return _CACHE["p"]


def _pack_wf(W, b):
    """[D,K,CI,co] + [D,co] -> [D, (66+1+66)*co] rows k0|bias|k1."""
    Wf = np.asarray(W, np.float32)
    bf = np.asarray(b, np.float32)
    d, k, ci, co = Wf.shape
    return np.ascontiguousarray(np.concatenate(
        [Wf[:, 0].reshape(d, ci * co), bf.reshape(d, co),
         Wf[:, 1].reshape(d, ci * co)], axis=1))


def kernel(X, state, E, gate_W, gate_b, upd_W, upd_b):
    X = np.asarray(X, np.float32)
    state = np.asarray(state, np.float32)
    E = np.asarray(E, np.float32)
    bf = ml_dtypes.bfloat16
    nc1, nc2 = _phases()
    cores = list(range(NCORES))

    efT = np.ascontiguousarray(E.T)                       # [16, 2048]
    xin = np.concatenate([X, state], -1)                  # [B, N, 66]
    xs1 = np.ascontiguousarray(
        xin.transpose(1, 0, 2).reshape(N, BC)).astype(bf)
    wf1 = _pack_wf(gate_W, gate_b)
    wf2 = _pack_wf(upd_W, upd_b)
    ones_nb = np.ones((1, NO, B), np.float32)

    in1, stl = [], []
    for c in cores:
        s = slice(c * NO, (c + 1) * NO)
        eoT = np.ascontiguousarray(E[s].T)                # [16, 256]
        xtc = np.ascontiguousarray(np.concatenate(
            [xin[:, s].transpose(2, 1, 0), ones_nb], 0)).astype(bf)
        stc = np.ascontiguousarray(
            state[:, s].transpose(2, 1, 0).reshape(HID, NO * B))
        stl.append(stc)
        in1.append(dict(EoT=eoT, EfT=efT, Wf=wf1, XS=xs1, XT=xtc, ST=stc))
    r1 = run_bass_kernel_spmd(nc1, in1, cores, trace=TRACE)
    res1 = r1.results

    # zs_full [m, b, c] from per-core zs [HID, NO*B] == (c, n, b)
    zs_all = np.concatenate(
        [r["zs"].reshape(HID, NO, B).transpose(1, 2, 0) for r in res1], 0)
    xc = np.concatenate([X.transpose(1, 0, 2), zs_all], 2)  # [N, B, 66]
    xs2 = np.ascontiguousarray(xc.reshape(N, BC)).astype(bf)
    in2 = []
    for c in cores:
        s = slice(c * NO, (c + 1) * NO)
        xtc = np.ascontiguousarray(np.concatenate(
            [xc[s].transpose(2, 0, 1), ones_nb], 0)).astype(bf)
        in2.append(dict(EoT=in1[c]["EoT"], EfT=efT, Wf=wf2,
                        XS=xs2, XT=xtc, ST=stl[c], R=res1[c]["r"]))
    r2 = run_bass_kernel_spmd(nc2, in2, cores, trace=TRACE)
    res2 = r2.results
    if TRACE:
        global LAST_EXEC_NS, LAST_PHASE_NS, LAST_TRACE
        LAST_PHASE_NS = [r1.exec_time_ns, r2.exec_time_ns]
        LAST_TRACE = [r1.instructions_and_trace, r2.instructions_and_trace]
        if r1.exec_time_ns and r2.exec_time_ns:
            LAST_EXEC_NS = r1.exec_time_ns + r2.exec_time_ns

    h = np.concatenate(
        [r["h"].reshape(HID, NO, B).transpose(2, 1, 0) for r in res2], 1)
    return np.ascontiguousarray(h, np.float32)
